# revision 10
# baseline (speedup 1.0000x reference)
"""Trainium2 Bass kernel for nn_CausalGraphLayer (gnn_message_passing).

Math: out[b,n,t,c] = tanh( sum_k w[c,n,k] * z[b, idx[n,k], t, c] )
      w[c,n,k] = (sum_nb coeff[c,nb] * bases[nb,n,k]) * adj[n,k]

Decomposition used here:
  A_nb[n,m]   = sum_k bases[nb,n,k]*adj[n,k]*[idx[n,k]==m]   (host-built, dense)
  Y_nb[n,tc]  = sum_m A_nb[n,m] * z_b[m,tc]                  (PE matmuls)
  out[n,t,c]  = tanh( sum_nb coeff[c,nb] * Y_nb[n,t,c] )     (DVE + ACT)

Sharding: 8 cores = 4 batches x 2 destination-node halves. Each core gets
z[b] (natural layout, no gather) plus its half's scatter matrices. A dense
formulation beats index-gather schemes here: gathering neighbor rows would
move ~134MB/core through DMA, while streaming z through the PE at full
128x128x512 utilization costs far less with ~13MB of DMA.

Default variant "bf16resf" (measured ~212us/core steady state, vs ~276us
for the f32r streaming baseline; rel err 2.8e-3 vs the 2e-2 gate):
 - A and z cast to bf16 on host (halves DMA; PE rate is 1 cycle/row for
   both bf16 and f32r, but bf16 emits a separate InstLdweights that can
   be deduplicated, while f32r matmuls are self-loading).
 - z[b] (8MB bf16) fully SBUF-resident; loops run (ni, half, nb, mc)
   with the 4 fj free-chunks of a half innermost, so 4 consecutive
   matmuls share one stationary A tile.
 - _dedup_ldweights removes the 3 redundant weight reloads per group
   (1024 -> 257 LDWs), taking the PE to its streaming roofline
   (1024 matmuls x 512 cols / 2.4GHz = 218us theoretical).
 - fj-major z layout: each arriving 1MB DMA chunk unlocks matmuls in
   every (ni,nb,mc) group, minimizing startup exposure.
 - PSUM as 2x 4-bank half-groups: DVE evacuation (x bc, +acc) of one
   group overlaps PE fill of the next; tanh on ACT, stores on the ACT
   ring.
"""

import sys

import numpy as np

B, N, T, C = 4, 1024, 64, 64
K_CURR, MAX_K, NUM_BASES = 16, 32, 4
TC = T * C  # 4096
HALF = N // 2  # 512 destination nodes per core
NCHUNK = HALF // 128  # 4 output row-chunks
MCHUNK = N // 128  # 8 contraction chunks
FREE = 512  # matmul moving free dim (one PSUM bank, fp32)
FCHUNK = TC // FREE  # 8 free chunks

_CACHE = {}

DEFAULT_VARIANT = "bf16resf"


def _import_concourse():
    try:
        import concourse.bass  # noqa: F401
    except ImportError:
        for p in ("/opt/trn_rl_repo", "/root/.axon_site/_ro/trn_rl_repo"):
            if p not in sys.path:
                sys.path.append(p)
        import concourse.bass  # noqa: F401


def _bf16():
    import ml_dtypes

    return np.dtype(ml_dtypes.bfloat16)


def _split_multi_waits(nc):
    """Split multi-sem waits into single-wait NOPs.

    The TPB ISA has one wait slot per instruction; the walrus build in this
    container errors with "Too many sync wait commands" on instructions
    carrying more than one SyncWait (Tile's tail drain does). Splitting into
    preceding same-engine NOPs is semantics-preserving: engine queues are
    FIFO and semaphores are monotone.
    """
    import concourse.mybir as mybir

    counter = [0]
    for fn in nc.m.functions:
        for bb in fn.blocks:
            new_insts = []
            changed = False
            for inst in bb.instructions:
                si = inst.sync_info
                if si is not None and si.on_wait and len(si.on_wait) > 1:
                    waits = list(si.on_wait)
                    for w in waits[:-1]:
                        counter[0] += 1
                        nop = mybir.InstNoOp(
                            name=f"WSPLIT-{counter[0]}", engine=inst.engine
                        )
                        nop.sync_info = mybir.SyncInfo(on_wait=[w], on_update=[])
                        new_insts.append(nop)
                    inst.sync_info = mybir.SyncInfo(
                        on_wait=[waits[-1]], on_update=list(si.on_update)
                    )
                    changed = True
                new_insts.append(inst)
            if changed:
                bb.instructions = new_insts
    return nc


def _dedup_ldweights(nc):
    """Drop InstLdweights that reload the exact weights already in the PE.

    bass emits one LDWEIGHTS per (bf16) matmul even when consecutive
    matmuls share the same stationary tile. The PE array state persists
    across matmuls, so a reload of an identical access pattern is pure
    overhead. Waits on a dropped LDW move to the next PE instruction
    (its matmul); LDWs carrying semaphore updates are kept.
    """
    import concourse.mybir as mybir

    removed = 0
    for fn in nc.m.functions:
        for bb in fn.blocks:
            new_insts = []
            pending_waits = []
            last_key = None
            for inst in bb.instructions:
                if getattr(inst, "engine", None) != mybir.EngineType.PE:
                    new_insts.append(inst)
                    continue
                tname = type(inst).__name__
                if tname == "InstLdweights":
                    si = inst.sync_info
                    has_upd = bool(si and si.on_update)
                    key = str(inst.ins[0])
                    if key == last_key and not has_upd:
                        if si and si.on_wait:
                            pending_waits.extend(si.on_wait)
                        removed += 1
                        continue
                    last_key = key
                elif tname == "InstMatmult":
                    if pending_waits:
                        si = inst.sync_info
                        waits = list(si.on_wait) if si else []
                        upds = list(si.on_update) if si else []
                        inst.sync_info = mybir.SyncInfo(
                            on_wait=waits + pending_waits, on_update=upds
                        )
                        pending_waits = []
                else:
                    # any other PE instruction invalidates tracking
                    last_key = None
                    if pending_waits:
                        si = inst.sync_info
                        waits = list(si.on_wait) if si else []
                        upds = list(si.on_update) if si else []
                        inst.sync_info = mybir.SyncInfo(
                            on_wait=waits + pending_waits, on_update=upds
                        )
                        pending_waits = []
                new_insts.append(inst)
            assert not pending_waits
            bb.instructions = new_insts
    return removed


def _build_stream(reps=1, dtype="f32r"):
    """Streaming-z structure: fj outer, z slices streamed, A resident.

    dtype="f32r": single-pass fp32 PE mode (1 cycle/row at free>=256).
    dtype="bf16": half the z/A DMA bytes; FWL-accelerated weight loads.
    """
    import concourse.bass as bass
    import concourse.mybir as mybir
    from concourse import tile

    f32 = mybir.dt.float32
    mm_dt = mybir.dt.float32r if dtype == "f32r" else mybir.dt.bfloat16

    nc = bass.Bass("TRN2", target_bir_lowering=False, debug=False)
    # z pre-rearranged on host to [fj, p, mc, f] so each tc-slice load is
    # fully contiguous (strided 2KB-piece loads ran at ~half DMA bandwidth).
    z_d = nc.dram_tensor("z", [FCHUNK, 128, MCHUNK, FREE], mm_dt, kind="ExternalInput")
    a_d = nc.dram_tensor(
        "a", [NCHUNK, 128, NUM_BASES, MCHUNK, 128], mm_dt, kind="ExternalInput"
    )
    b_d = nc.dram_tensor("bc", [NUM_BASES, 128, FREE], f32, kind="ExternalInput")
    o_d = nc.dram_tensor("out", [HALF, TC], f32, kind="ExternalOutput")

    with tile.TileContext(nc) as tc:
        with (
            tc.tile_pool(name="zp", bufs=3) as zp,
            tc.tile_pool(name="apool", bufs=1) as apool,
            tc.tile_pool(name="bp", bufs=1) as bp,
            tc.tile_pool(name="psum", bufs=2, space="PSUM") as psp,
            tc.tile_pool(name="mul", bufs=2) as mulp,
            tc.tile_pool(name="sum", bufs=2) as sump,
            tc.tile_pool(name="outp", bufs=3) as outp,
        ):
            bc = bp.tile([128, NUM_BASES, FREE], f32)
            nc.sync.dma_start(out=bc[:], in_=b_d.ap().rearrange("nb p f -> p nb f"))

            # All four A row-chunk slabs stay resident (fj-outer revisits
            # every ni per fj). Loaded lazily at first use so the first z
            # slice isn't queued behind the A data.
            a_ts = {}

            def get_a(ni):
                if ni not in a_ts:
                    a_t = apool.tile(
                        [128, NUM_BASES, MCHUNK, 128], mm_dt, tag=f"a{ni}"
                    )
                    nc.sync.dma_start(out=a_t[:], in_=a_d[ni])
                    a_ts[ni] = a_t
                return a_ts[ni]

            for _rep in range(reps):
              for fj in range(FCHUNK):
                # One tc-slice of z covering ALL contraction chunks: the
                # first matmul groups can complete after ~6us instead of
                # waiting for the full z load. Never reused after this
                # fj -> streaming pool.
                z_t = zp.tile([128, MCHUNK, FREE], mm_dt, tag="z")
                # SWDGE (gpsimd) load: HWDGE loads on the sync ring failed to
                # overlap PE here (~fully serial z-DMA); SWDGE pipelines.
                nc.gpsimd.dma_start(out=z_t[:], in_=z_d[fj])
                for ni in range(NCHUNK):
                    ps = psp.tile([128, NUM_BASES, FREE], f32, tag="ps")
                    for nb in range(NUM_BASES):
                        for mc in range(MCHUNK):
                            nc.tensor.matmul(
                                ps[:, nb, :],
                                get_a(ni)[:, nb, mc, :],
                                z_t[:, mc, :],
                                start=(mc == 0),
                                stop=(mc == MCHUNK - 1),
                            )
                    m = mulp.tile([128, NUM_BASES, FREE], f32, tag="m")
                    nc.vector.tensor_mul(m[:], ps[:], bc[:])
                    s = sump.tile([128, 2, FREE], f32, tag="s")
                    nc.vector.tensor_add(s[:], m[:, 0:2, :], m[:, 2:4, :])
                    out_t = outp.tile([128, FREE], f32, tag="o")
                    acc = sump.tile([128, FREE], f32, tag="acc")
                    nc.vector.tensor_add(acc[:], s[:, 0, :], s[:, 1, :])
                    nc.scalar.activation(
                        out_t[:], acc[:], mybir.ActivationFunctionType.Tanh
                    )
                    # Stores issue from the ACT ring: HWDGE DMAs are FIFO per
                    # issuing engine, and a store's wait-on-tanh would block
                    # the SP ring where the z prefetches live.
                    nc.scalar.dma_start(
                        out=o_d[bass.ts(ni, 128), bass.ts(fj, FREE)], in_=out_t[:]
                    )

    _split_multi_waits(nc)
    return nc


def _build_resident(reps=1, dedup=False, fjmajor=False):
    """bf16, z fully SBUF-resident, fj-inner loops.

    Consecutive matmuls in the inner fj loop share the same stationary A
    tile, so the per-matmul weight reload can be elided/hidden. PSUM is
    split into two 4-bank half-groups so the DVE evacuation of one group
    overlaps the PE fill of the next.
    """
    import concourse.bass as bass
    import concourse.mybir as mybir
    from concourse import tile

    f32 = mybir.dt.float32
    bf16 = mybir.dt.bfloat16
    FH = FCHUNK // 2  # 4 fj per half-group

    nc = bass.Bass("TRN2", target_bir_lowering=False, debug=False)
    # z loaded once, stays resident (8MB). fj-major layout streams in
    # 1MB fj-chunks each of which unlocks matmuls across every (ni,nb,mc),
    # shrinking the startup DMA exposure vs mc-major.
    zshape = (
        [128, FCHUNK, MCHUNK, FREE] if fjmajor else [128, MCHUNK, FCHUNK, FREE]
    )
    z_d = nc.dram_tensor("z", zshape, bf16, kind="ExternalInput")
    a_d = nc.dram_tensor(
        "a", [NCHUNK, 128, NUM_BASES, MCHUNK, 128], bf16, kind="ExternalInput"
    )
    # bc replicated over the 4 fj slots of a half-group: [nb, p, FH*FREE]
    b_d = nc.dram_tensor("bc", [NUM_BASES, 128, FH * FREE], f32, kind="ExternalInput")
    o_d = nc.dram_tensor("out", [HALF, TC], f32, kind="ExternalOutput")

    with tile.TileContext(nc) as tc:
        with (
            tc.tile_pool(name="zp", bufs=1) as zp,
            tc.tile_pool(name="apool", bufs=1) as apool,
            tc.tile_pool(name="bp", bufs=1) as bp,
            tc.tile_pool(name="psum", bufs=2, space="PSUM") as psp,
            tc.tile_pool(name="acc", bufs=2) as accp,
            tc.tile_pool(name="tmp", bufs=2) as tmpp,
            tc.tile_pool(name="outp", bufs=2) as outp,
        ):
            bc = bp.tile([128, NUM_BASES, FH, FREE], f32)
            nc.sync.dma_start(out=bc[:], in_=b_d.ap().rearrange("nb p f -> p nb f"))

            zfull = zp.tile(zshape, bf16, tag="z")
            for c0 in range(zshape[1]):
                nc.gpsimd.dma_start(
                    out=zfull[:, c0, :, :], in_=z_d[:, c0, :, :]
                )

            def zmov(mc, fj):
                return zfull[:, fj, mc, :] if fjmajor else zfull[:, mc, fj, :]

            a_ts = {}

            def get_a(ni):
                if ni not in a_ts:
                    a_t = apool.tile(
                        [128, NUM_BASES, MCHUNK, 128], bf16, tag=f"a{ni}"
                    )
                    nc.sync.dma_start(out=a_t[:], in_=a_d[ni])
                    a_ts[ni] = a_t
                return a_ts[ni]

            for _rep in range(reps):
              for ni in range(NCHUNK):
                for half in range(2):
                    acc = accp.tile([128, FH, FREE], f32, tag="acc")
                    for nb in range(NUM_BASES):
                        ps = psp.tile([128, FH, FREE], f32, tag="ps")
                        for mc in range(MCHUNK):
                            for j in range(FH):
                                fj = half * FH + j
                                nc.tensor.matmul(
                                    ps[:, j, :],
                                    get_a(ni)[:, nb, mc, :],
                                    zmov(mc, fj),
                                    start=(mc == 0),
                                    stop=(mc == MCHUNK - 1),
                                )
                        if nb == 0:
                            nc.vector.tensor_mul(acc[:], ps[:], bc[:, 0, :, :])
                        else:
                            m = tmpp.tile([128, FH, FREE], f32, tag="m")
                            nc.vector.tensor_mul(m[:], ps[:], bc[:, nb, :, :])
                            nc.vector.tensor_add(acc[:], acc[:], m[:])
                    out_t = outp.tile([128, FH, FREE], f32, tag="o")
                    nc.scalar.activation(
                        out_t[:], acc[:], mybir.ActivationFunctionType.Tanh
                    )
                    nc.scalar.dma_start(
                        out=o_d[
                            bass.ts(ni, 128), bass.ts(half, FH * FREE)
                        ],
                        in_=out_t[:],
                    )

    if dedup:
        _dedup_ldweights(nc)
    _split_multi_waits(nc)
    return nc


def _prep_common(z, neighbor_indices, adjacency, basis_weights, channel_coeffs):
    z = np.asarray(z, dtype=np.float32)
    idx = np.asarray(neighbor_indices)
    k = idx.shape[1]
    if k > adjacency.shape[1]:
        idx = idx[:, : adjacency.shape[1]]
        k = adjacency.shape[1]
    adj = np.asarray(adjacency, dtype=np.float32)[:, :k]
    bases = np.asarray(basis_weights, dtype=np.float32)[:, :, :k]
    coeff = np.asarray(channel_coeffs, dtype=np.float32)

    abases = bases * adj[None, :, :]  # (NB, N, k)

    a_list = []
    for h in range(2):
        rows = slice(h * HALF, (h + 1) * HALF)
        idx_h = idx[rows]  # (HALF, k)
        # A_T[nb, m, nl] = sum_k abases[nb, n, k] over idx[n,k]==m
        a_t = np.zeros((NUM_BASES, N, HALF), dtype=np.float32)
        cols = np.repeat(np.arange(HALF), k)
        flat_idx = idx_h.ravel()
        for nb in range(NUM_BASES):
            np.add.at(a_t[nb], (flat_idx, cols), abases[nb, rows].ravel())
        # (ni, p, nb, mc, nl) with m = mc*128+p, n_local = ni*128+nl
        a_re = np.ascontiguousarray(
            a_t.reshape(NUM_BASES, MCHUNK, 128, NCHUNK, 128).transpose(3, 2, 0, 1, 4)
        )
        a_list.append(a_re)
    return z, coeff, a_list


def _host_prep_stream(dtype, **inputs):
    z, coeff, a_list = _prep_common(**inputs)
    np_dt = np.float32 if dtype == "f32r" else _bf16()

    # bc[nb, p, f] = coeff[f % C, nb] (partition-replicated free-dim pattern)
    bc = np.ascontiguousarray(
        np.broadcast_to(
            np.tile(coeff.T[:, None, :], (1, 1, FREE // C)),
            (NUM_BASES, 128, FREE),
        ).reshape(NUM_BASES, 128, FREE)
    ).astype(np.float32)

    in_maps = []
    for core in range(8):
        b, h = divmod(core, 2)
        z_re = np.ascontiguousarray(
            z[b].reshape(MCHUNK, 128, FCHUNK, FREE).transpose(2, 1, 0, 3)
        ).astype(np_dt)
        in_maps.append({"z": z_re, "a": a_list[h].astype(np_dt), "bc": bc})
    return in_maps


def _host_prep_resident(fjmajor=False, **inputs):
    z, coeff, a_list = _prep_common(**inputs)
    bf = _bf16()
    FH = FCHUNK // 2

    bc1 = np.tile(coeff.T[:, None, :], (1, 1, FREE // C))  # (NB, 1, FREE)
    bc = np.ascontiguousarray(
        np.broadcast_to(bc1[:, :, None, :], (NUM_BASES, 128, FH, FREE)).reshape(
            NUM_BASES, 128, FH * FREE
        )
    ).astype(np.float32)

    in_maps = []
    for core in range(8):
        b, h = divmod(core, 2)
        perm = (1, 2, 0, 3) if fjmajor else (1, 0, 2, 3)
        z_re = np.ascontiguousarray(
            z[b].reshape(MCHUNK, 128, FCHUNK, FREE).transpose(perm)
        ).astype(bf)
        in_maps.append({"z": z_re, "a": a_list[h].astype(bf), "bc": bc})
    return in_maps


def _build_mmb(reps=1, dedup=False, groups=16):
    """PE microbenchmark: the resd-style matmul stream with no DVE/ACT/DMA
    per group beyond one PSUM evacuation copy. Measures per-MM cost and
    LDWEIGHTS exposure on this silicon."""
    import concourse.bass as bass
    import concourse.mybir as mybir
    from concourse import tile

    f32 = mybir.dt.float32
    bf16 = mybir.dt.bfloat16

    nc = bass.Bass("TRN2", target_bir_lowering=False, debug=False)
    z_d = nc.dram_tensor("z", [128, MCHUNK, FREE], bf16, kind="ExternalInput")
    a_d = nc.dram_tensor("a", [128, NUM_BASES, MCHUNK, 128], bf16,
                         kind="ExternalInput")
    o_d = nc.dram_tensor("out", [128, FREE], f32, kind="ExternalOutput")

    with tile.TileContext(nc) as tc:
        with (
            tc.tile_pool(name="zp", bufs=1) as zp,
            tc.tile_pool(name="ap", bufs=1) as ap,
            tc.tile_pool(name="psum", bufs=2, space="PSUM") as psp,
            tc.tile_pool(name="sp", bufs=2) as sp,
        ):
            z_t = zp.tile([128, MCHUNK, FREE], bf16)
            nc.gpsimd.dma_start(out=z_t[:], in_=z_d.ap())
            a_t = ap.tile([128, NUM_BASES, MCHUNK, 128], bf16)
            nc.sync.dma_start(out=a_t[:], in_=a_d.ap())
            s_last = None
            for _rep in range(reps):
                for g in range(groups):
                    ps = psp.tile([128, 4, FREE], f32, tag="ps")
                    for mc in range(MCHUNK):
                        for j in range(4):
                            nc.tensor.matmul(
                                ps[:, j, :],
                                a_t[:, g % NUM_BASES, mc, :],
                                z_t[:, mc, :],
                                start=(mc == 0),
                                stop=(mc == MCHUNK - 1),
                            )
                    s = sp.tile([128, FREE], f32, tag="s")
                    nc.vector.tensor_add(s[:], ps[:, 0, :], ps[:, 2, :])
                    s_last = s
            nc.vector.tensor_copy(out=s_last[:], in_=s_last[:])
            nc.sync.dma_start(out=o_d.ap(), in_=s_last[:])

    if dedup:
        _dedup_ldweights(nc)
    _split_multi_waits(nc)
    return nc


def _host_prep_mmb(**inputs):
    z, coeff, a_list = _prep_common(**inputs)
    bf = _bf16()
    z_re = np.ascontiguousarray(
        z[0].reshape(MCHUNK, 128, FCHUNK, FREE)[:, :, 0, :].transpose(1, 0, 2)
    ).astype(bf)
    a_re = a_list[0][0].astype(bf)
    return [{"z": z_re, "a": a_re} for _ in range(8)]


PSEUDO = {"mmb", "mmbd"}

VARIANTS = {
    "f32r": (lambda reps=1: _build_stream(reps, "f32r"),
             lambda **i: _host_prep_stream("f32r", **i)),
    "bf16": (lambda reps=1: _build_stream(reps, "bf16"),
             lambda **i: _host_prep_stream("bf16", **i)),
    "bf16res": (_build_resident, _host_prep_resident),
    "bf16resd": (lambda reps=1: _build_resident(reps, dedup=True),
                 _host_prep_resident),
    "bf16resf": (lambda reps=1: _build_resident(reps, dedup=True, fjmajor=True),
                 lambda **i: _host_prep_resident(fjmajor=True, **i)),
    "mmb": (lambda reps=1: _build_mmb(reps, dedup=False), _host_prep_mmb),
    "mmbd": (lambda reps=1: _build_mmb(reps, dedup=True), _host_prep_mmb),
}


def build_variant(name, reps=1):
    key = ("nc", name, reps)
    if key not in _CACHE:
        _import_concourse()
        _CACHE[key] = VARIANTS[name][0](reps=reps)
    return _CACHE[key]


def host_prep_variant(name, **inputs):
    return VARIANTS[name][1](**inputs)


def assemble_output(name, results):
    out = np.empty((B, N, T, C), dtype=np.float32)
    for core in range(8):
        b, h = divmod(core, 2)
        out[b, h * HALF : (h + 1) * HALF] = results[core]["out"].reshape(HALF, T, C)
    return out


# ---- legacy aliases used by test.py ----

def _build_program(reps=1):
    return VARIANTS[DEFAULT_VARIANT][0](reps=reps)


def _host_prep(**inputs):
    return host_prep_variant(DEFAULT_VARIANT, **inputs)


def _get_program(reps=1):
    return build_variant(DEFAULT_VARIANT, reps)


def run_on_hw(in_maps, **kwargs):
    from concourse.bass_utils import run_bass_kernel_spmd

    nc = _get_program()
    return run_bass_kernel_spmd(nc, in_maps, core_ids=list(range(8)), **kwargs)


def kernel(z, neighbor_indices, adjacency, basis_weights, channel_coeffs):
    _import_concourse()
    in_maps = _host_prep(
        z=z,
        neighbor_indices=neighbor_indices,
        adjacency=adjacency,
        basis_weights=basis_weights,
        channel_coeffs=channel_coeffs,
    )
    res = run_on_hw(in_maps)
    return assemble_output(DEFAULT_VARIANT, res.results)
